# revision 24
# baseline (speedup 1.0000x reference)
"""Trainium2 Bass kernel for BayesianChangePointDetector (segment_reduce).

Contract: kernel(**inputs) takes FULL inputs (x:[128,8192,32] f32, plus 3
scalar prior params) and returns the FULL [128] f32 output. Internally the
batch dim is sharded across 8 NeuronCores (16 rows each, pure data parallel,
no collectives), each core runs the same Bass/Tile program, and the host
concatenates the 8 per-core [16] outputs.

Per-core layout: partition p in [0,128) owns t in [64p, 64p+64); the free dim
is (b, u) with b in [0,16) batch rows and u in [0,64). The heavy pass is a
single DVE reduce over N=32; prefix sums use the native tensor_tensor_scan
plus a cross-partition carry fixed up with a triangular-ones matmul on PE.
"""

import sys

if "/opt/trn_rl_repo" not in sys.path:
    sys.path.insert(0, "/opt/trn_rl_repo")

import math
from contextlib import ExitStack

import numpy as np

import concourse.bass as bass
import concourse.tile as tile
from concourse import mybir

F32 = mybir.dt.float32
AF = mybir.ActivationFunctionType
ALU = mybir.AluOpType
AX = mybir.AxisListType

B, T, N = 128, 8192, 32
NCORES = 8
BL = B // NCORES  # 16 batch rows per core
P = 128           # partitions = t-blocks
U = T // P        # 64 t's per partition
BC = 4            # batch rows per processing chunk
NCHUNK = BL // BC
NS = 32           # scalar-slot count
NEG = -1.0e30

# near-end threshold: P_split > 6553  <=>  g >= 6553 (g = P_split-1 = 64p+u)
NE_P0 = 6553 // U          # 102
NE_U0 = 6553 - NE_P0 * U   # 25
# valid candidates: P_split in [16, 8176) <=> g in [15, 8175)
LO_INV_U = 15              # g<15 -> p==0, u<15 invalid
HI_INV_U = 8174 - 127 * U + 1  # g>8174 -> p==127, u>=47 invalid


def build_body(ctx, tc, x, params, gvec, utc, onesc, idc, out):
    nc = tc.nc
    pers = ctx.enter_context(tc.tile_pool(name="pers", bufs=1))
    xp = ctx.enter_context(tc.tile_pool(name="xp", bufs=2))
    wk = ctx.enter_context(tc.tile_pool(name="wk", bufs=2))
    psp = ctx.enter_context(tc.tile_pool(name="psp", bufs=2, space="PSUM"))
    ps1 = ctx.enter_context(tc.tile_pool(name="ps1", bufs=1, space="PSUM"))

    # ---------- persistent tiles ----------
    ut_t = pers.tile([P, P], F32)     # strictly-upper triangular ones (q<m)
    ones_t = pers.tile([P, P], F32)   # all-ones
    id_t = pers.tile([P, P], F32)     # identity (PE transpose)
    gt = pers.tile([P, U], F32)       # g = 64p+u
    nc.sync.dma_start(ut_t[:], utc[:])
    nc.sync.dma_start(ones_t[:], onesc[:])
    nc.sync.dma_start(id_t[:], idc[:])
    nc.sync.dma_start(gt[:], gvec[:])

    ptile = pers.tile([P, 3], F32)
    nc.sync.dma_start(ptile[:], params[:])

    # scalar slots, computed redundantly on all 128 partitions
    sv = pers.tile([P, NS], F32)
    tmp = pers.tile([P, 8], F32)

    def s(i):
        return sv[:, i : i + 1]

    def tm(i):
        return tmp[:, i : i + 1]

    # ---------- scalar prep on partition 0 ----------
    # slots: 0 pm, 1 inv_nv, 2 inv_pv, 3 neg_inv_nv, 4 zRb, 5 k, 6 c,
    # 7 -kq/2, 8 k^2/2, 9 kq/2, 10 c*k, 11 c^2/2, 12 sc, 13 pvW,
    # 14 L2pinv, 15 Lpv, 16 LpvW, 17 8192*inv_nv, 18 inv_nv/8192,
    # 19 bfWc, 20 pv, 21 nv, 22 pm^2*inv_pv, 23 -4096*L2pinv
    # softplus(x) = ln(1 + exp(x)); Exp+Ln share one ACT table set
    nc.scalar.activation(tm(0), ptile[:, 1:2], AF.Exp)
    nc.vector.tensor_scalar_add(tm(0), tm(0), 1.0)
    nc.scalar.activation(s(20), tm(0), AF.Ln)
    nc.scalar.activation(tm(1), ptile[:, 2:3], AF.Exp)
    nc.vector.tensor_scalar_add(tm(1), tm(1), 1.0)
    nc.scalar.activation(s(21), tm(1), AF.Ln)
    nc.vector.tensor_copy(s(0), ptile[:, 0:1])
    nc.vector.reciprocal(s(1), s(21))
    nc.vector.reciprocal(s(2), s(20))
    nc.vector.tensor_scalar_mul(s(3), s(1), -1.0)
    nc.vector.tensor_scalar(s(4), s(1), 8191.0, s(2), ALU.mult, ALU.add)
    nc.vector.tensor_scalar_mul(s(5), s(1), 1.0 / 32.0)
    nc.vector.tensor_mul(s(6), s(0), s(2))
    nc.vector.tensor_scalar_mul(s(7), s(1), -0.5 / 1024.0)
    nc.vector.tensor_scalar_mul(s(9), s(1), 0.5 / 1024.0)
    nc.vector.tensor_mul(tm(0), s(5), s(5))
    nc.vector.tensor_scalar_mul(s(8), tm(0), 0.5)
    nc.vector.tensor_mul(s(10), s(6), s(5))
    nc.vector.tensor_mul(tm(1), s(6), s(6))
    nc.vector.tensor_scalar_mul(s(11), tm(1), 0.5)
    nc.scalar.activation(s(14), s(21), AF.Ln, scale=2.0 * math.pi)
    nc.scalar.activation(s(15), s(20), AF.Ln)
    nc.vector.tensor_scalar_mul(s(17), s(1), 8192.0)
    nc.vector.tensor_scalar(tm(2), s(1), 8192.0, s(2), ALU.mult, ALU.add)
    nc.vector.reciprocal(s(13), tm(2))
    nc.scalar.activation(s(16), s(13), AF.Ln)
    nc.vector.tensor_scalar_mul(s(18), s(1), 1.0 / 8192.0)
    nc.vector.tensor_mul(tm(3), s(0), s(0))
    nc.vector.tensor_mul(s(22), tm(3), s(2))
    nc.vector.tensor_scalar_mul(s(23), s(14), -4096.0)
    nc.vector.tensor_sub(tm(4), s(23), s(15))
    nc.vector.tensor_sub(s(12), tm(4), s(22))
    nc.vector.tensor_sub(tm(5), s(16), s(15))
    nc.vector.tensor_scalar_mul(tm(5), tm(5), 0.5)
    nc.vector.tensor_add(tm(6), s(23), tm(5))
    nc.vector.tensor_scalar_mul(tm(7), s(22), -0.5)
    nc.vector.tensor_add(s(19), tm(6), tm(7))

    def sb(i, np_=P, p0=0):
        return sv[p0 : p0 + np_, i : i + 1]

    # ---------- per-candidate coefficient vectors [P, U] ----------
    nf = pers.tile([P, U], F32)
    nc.vector.tensor_scalar_add(nf[:], gt[:], 1.0)
    zL = pers.tile([P, U], F32)
    nc.vector.tensor_scalar(zL[:], nf[:], sb(1), sb(2), ALU.mult, ALU.add)
    pvnL = pers.tile([P, U], F32)
    nc.vector.reciprocal(pvnL[:], zL[:])
    zR = pers.tile([P, U], F32)
    nc.vector.tensor_scalar(zR[:], gt[:], sb(3), sb(4), ALU.mult, ALU.add)
    pvnR = pers.tile([P, U], F32)
    nc.vector.reciprocal(pvnR[:], zR[:])
    lpvnL = pers.tile([P, U], F32)
    nc.scalar.activation(lpvnL[:], pvnL[:], AF.Ln)
    lpvnR = pers.tile([P, U], F32)
    nc.scalar.activation(lpvnR[:], pvnR[:], AF.Ln)
    kc2 = pers.tile([P, U], F32)
    nc.vector.tensor_add(kc2[:], lpvnL[:], lpvnR[:])

    nRf = pers.tile([P, U], F32)
    nc.vector.tensor_scalar(nRf[:], gt[:], -1.0, 8191.0, ALU.mult, ALU.add)
    gc = pers.tile([P, U], F32)
    nc.vector.tensor_scalar_max(gc[:], gt[:], 1.0)
    inv_n1 = pers.tile([P, U], F32)
    nc.vector.reciprocal(inv_n1[:], gc[:])
    nR1c = pers.tile([P, U], F32)
    nc.vector.tensor_scalar(nR1c[:], gt[:], -1.0, 8190.0, ALU.mult, ALU.add)
    nc.vector.tensor_scalar_max(nR1c[:], nR1c[:], 1.0)
    inv_nR1 = pers.tile([P, U], F32)
    nc.vector.reciprocal(inv_nR1[:], nR1c[:])
    inv_n = pers.tile([P, U], F32)
    nc.vector.reciprocal(inv_n[:], nf[:])
    inv_nR = pers.tile([P, U], F32)
    nRc = pers.tile([P, U], F32)
    nc.vector.tensor_scalar_max(nRc[:], nRf[:], 1.0)
    nc.vector.reciprocal(inv_nR[:], nRc[:])

    n_n1 = pers.tile([P, U], F32)
    nc.vector.tensor_mul(n_n1[:], nf[:], inv_n1[:])
    nR_nR1 = pers.tile([P, U], F32)
    nc.vector.tensor_mul(nR_nR1[:], nRf[:], inv_nR1[:])
    i_nn1 = pers.tile([P, U], F32)
    nc.vector.tensor_mul(i_nn1[:], inv_n[:], inv_n1[:])
    i_nRnR1 = pers.tile([P, U], F32)
    nc.vector.tensor_mul(i_nRnR1[:], inv_nR[:], inv_nR1[:])

    CBL = pers.tile([P, U], F32)
    nc.scalar.activation(CBL[:], n_n1[:], AF.Copy, scale=sb(7))
    CBR = pers.tile([P, U], F32)
    nc.scalar.activation(CBR[:], nR_nR1[:], AF.Copy, scale=sb(7))
    # CA2L = 0.5*kq*i_nn1 + 0.5*k^2*pvnL
    CA2L = pers.tile([P, U], F32)
    q1 = pers.tile([P, U], F32)
    nc.scalar.activation(q1[:], pvnL[:], AF.Copy, scale=sb(8))
    q2 = pers.tile([P, U], F32)
    nc.scalar.activation(q2[:], i_nn1[:], AF.Copy, scale=sb(9))
    nc.vector.tensor_add(CA2L[:], q1[:], q2[:])
    CA2R = pers.tile([P, U], F32)
    q1b = pers.tile([P, U], F32)
    nc.scalar.activation(q1b[:], pvnR[:], AF.Copy, scale=sb(8))
    q2b = pers.tile([P, U], F32)
    nc.scalar.activation(q2b[:], i_nRnR1[:], AF.Copy, scale=sb(9))
    nc.vector.tensor_add(CA2R[:], q1b[:], q2b[:])
    CAL = pers.tile([P, U], F32)
    nc.scalar.activation(CAL[:], pvnL[:], AF.Copy, scale=sb(10))
    CAR = pers.tile([P, U], F32)
    nc.scalar.activation(CAR[:], pvnR[:], AF.Copy, scale=sb(10))
    Cc = pers.tile([P, U], F32)
    p12 = pers.tile([P, U], F32)
    nc.vector.tensor_add(p12[:], pvnL[:], pvnR[:])
    cc1 = pers.tile([P, U], F32)
    nc.scalar.activation(cc1[:], p12[:], AF.Copy, scale=sb(11))
    cct = pers.tile([P, U], F32)
    nc.vector.tensor_scalar(cct[:], kc2[:], 0.5, sb(12), ALU.mult, ALU.add)
    nc.vector.tensor_add(Cc[:], cc1[:], cct[:])
    # bake the invalid-candidate mask into Cc: bf = ... + Cc ~ -1e30 there.
    # valid g in [15, 8175); compute via two is_ge comparisons on gt.
    mlo = pers.tile([P, U], F32)
    nc.vector.tensor_scalar(mlo[:], gt[:], 14.5, NEG, ALU.is_lt, ALU.mult)
    mhi = pers.tile([P, U], F32)
    nc.vector.tensor_scalar(mhi[:], gt[:], 8174.5, NEG, ALU.is_ge, ALU.mult)
    nc.vector.tensor_add(Cc[:], Cc[:], mlo[:])
    nc.vector.tensor_add(Cc[:], Cc[:], mhi[:])
    # near-end 0/1 mask (g >= 6553)
    nemask = pers.tile([P, U], F32)
    nc.vector.tensor_scalar(nemask[:], gt[:], 6552.5, None, ALU.is_ge)

    # ---------- persistent accumulators ----------
    bund = pers.tile([P, 80], F32)  # [0:16) rmax | [16:32) Zp | [32:48) En | [48:64) At | [64:80) Bt
    zeros = pers.tile([P, BC * U], F32)
    nc.gpsimd.memset(zeros[:], 0.0)

    # ---------- per-chunk pipeline ----------
    for ci in range(NCHUNK):
        bs = ci * BC
        xt = xp.tile([P, BC, U, N], F32)
        src = x[bs : bs + BC].rearrange("b (p u) n -> p b u n", p=P)
        nc.sync.dma_start(xt[:], src)

        sr = wk.tile([P, BC, U], F32)
        nc.vector.tensor_reduce(sr[:], xt[:].rearrange("p b u n -> p (b u) n"), AX.X, ALU.add)
        sq = wk.tile([P, BC, U], F32)
        nc.scalar.activation(sq[:], sr[:], AF.Square)

        A = wk.tile([P, BC, U], F32)
        nc.vector.tensor_tensor_scan(
            A[:].rearrange("p b u -> p (b u)"),
            sr[:].rearrange("p b u -> p (b u)"),
            zeros[:],
            0.0,
            ALU.add,
            ALU.add,
        )
        Bt_ = wk.tile([P, BC, U], F32)
        nc.vector.tensor_tensor_scan(
            Bt_[:].rearrange("p b u -> p (b u)"),
            sq[:].rearrange("p b u -> p (b u)"),
            zeros[:],
            0.0,
            ALU.add,
            ALU.add,
        )

        # carry fix: rowprev, chunk totals, triangular matmul
        rv = wk.tile([P, 2 * BC], F32)  # [0:BC) rvA | [BC:2BC) rvB
        nc.gpsimd.memset(rv[:, 0:1], 0.0)
        nc.gpsimd.memset(rv[:, BC : BC + 1], 0.0)
        nc.vector.tensor_copy(rv[:, 1:BC], A[:, 0 : BC - 1, U - 1])
        nc.vector.tensor_copy(rv[:, BC + 1 : 2 * BC], Bt_[:, 0 : BC - 1, U - 1])
        ct = wk.tile([P, 2 * BC], F32)
        nc.vector.tensor_sub(ct[:, 0:BC], A[:, :, U - 1], rv[:, 0:BC])
        nc.vector.tensor_sub(ct[:, BC : 2 * BC], Bt_[:, :, U - 1], rv[:, BC : 2 * BC])
        g_ps = psp.tile([P, 2 * BC], F32)
        nc.tensor.matmul(g_ps[:], ut_t[:], ct[:])
        tot_ps = psp.tile([P, 2 * BC], F32)
        nc.tensor.matmul(tot_ps[:], ones_t[:], ct[:])
        off = wk.tile([P, 2 * BC], F32)
        nc.vector.tensor_sub(off[:], g_ps[:], rv[:])

        offA_b = off[:, 0:BC].unsqueeze(2).broadcast_to([P, BC, U])
        offB_b = off[:, BC : 2 * BC].unsqueeze(2).broadcast_to([P, BC, U])
        nc.vector.tensor_add(A[:], A[:], offA_b)
        nc.vector.tensor_add(Bt_[:], Bt_[:], offB_b)

        At_b = tot_ps[:, 0:BC].unsqueeze(2).broadcast_to([P, BC, U])
        Btot_b = tot_ps[:, BC : 2 * BC].unsqueeze(2).broadcast_to([P, BC, U])
        AR = wk.tile([P, BC, U], F32)
        nc.vector.scalar_tensor_tensor(AR[:], A[:], -1.0, At_b, ALU.mult, ALU.add)
        BR = wk.tile([P, BC, U], F32)
        nc.vector.scalar_tensor_tensor(BR[:], Bt_[:], -1.0, Btot_b, ALU.mult, ALU.add)

        A2 = wk.tile([P, BC, U], F32)
        nc.scalar.activation(A2[:], A[:], AF.Square)
        AR2 = wk.tile([P, BC, U], F32)
        nc.scalar.activation(AR2[:], AR[:], AF.Square)

        def cb(t):
            return t[:].unsqueeze(1).broadcast_to([P, BC, U])

        bf = wk.tile([P, BC, U], F32)
        p2 = wk.tile([P, BC, U], F32)
        p3 = wk.tile([P, BC, U], F32)
        p4 = wk.tile([P, BC, U], F32)
        p5 = wk.tile([P, BC, U], F32)
        p6 = wk.tile([P, BC, U], F32)
        nc.vector.tensor_mul(bf[:], A[:], cb(CAL))
        nc.vector.tensor_mul(p2[:], A2[:], cb(CA2L))
        nc.vector.tensor_mul(p3[:], Bt_[:], cb(CBL))
        nc.vector.tensor_mul(p4[:], AR[:], cb(CAR))
        nc.vector.tensor_mul(p5[:], AR2[:], cb(CA2R))
        nc.vector.tensor_mul(p6[:], BR[:], cb(CBR))
        nc.gpsimd.tensor_add(bf[:], bf[:], p2[:])
        nc.gpsimd.tensor_add(bf[:], bf[:], p3[:])
        nc.gpsimd.tensor_add(bf[:], bf[:], p4[:])
        nc.gpsimd.tensor_add(bf[:], bf[:], p5[:])
        nc.gpsimd.tensor_add(bf[:], bf[:], p6[:])
        nc.gpsimd.tensor_add(bf[:], bf[:], cb(Cc))

        # per-(p,b) max, exp with shift, partial sums
        nc.vector.tensor_reduce(bund[:, bs : bs + BC], bf[:], AX.X, ALU.max)
        negr = wk.tile([P, BC], F32)
        nc.vector.tensor_scalar_mul(negr[:], bund[:, bs : bs + BC], -1.0)
        e = wk.tile([P, BC, U], F32)
        escr = wk.tile([P, U], F32)
        for b in range(BC):
            nc.scalar.activation(
                e[:, b, :],
                bf[:, b, :],
                AF.Exp,
                bias=negr[:, b : b + 1],
                accum_out=bund[:, 16 + bs + b : 17 + bs + b],
            )
            # near-end partial sum: sum_u e * nemask
            nc.vector.scalar_tensor_tensor(
                escr[:],
                e[:, b, :],
                1.0,
                nemask[:],
                ALU.mult,
                ALU.mult,
                accum_out=bund[:, 32 + bs + b : 33 + bs + b],
            )
        nc.scalar.copy(bund[:, 48 + bs : 48 + bs + BC], tot_ps[:, 0:BC])
        nc.scalar.copy(bund[:, 64 + bs : 64 + bs + BC], tot_ps[:, BC : 2 * BC])

    # ---------- finale: per-group PE transposes (keep partition base 0) ----------
    Tall = ps1.tile([BL, 5, P], F32)
    for k in range(5):
        nc.tensor.transpose(Tall[:, k, :], bund[:, 16 * k : 16 * k + BL], id_t[:])
    Tm = Tall[:, 0, :]
    Tz = Tall[:, 1, :]
    Te = Tall[:, 2, :]
    Ta = Tall[:, 3, :]
    Tb = Tall[:, 4, :]

    M16 = pers.tile([BL, 1], F32)
    nc.vector.tensor_reduce(M16[:], Tm[:], AX.X, ALU.max)
    d = pers.tile([BL, P], F32)
    nc.vector.tensor_scalar_sub(d[:], Tm[:], M16[:])
    w = pers.tile([BL, P], F32)
    nc.scalar.activation(w[:], d[:], AF.Exp)
    wz = pers.tile([BL, P], F32)
    Zb = pers.tile([BL, 1], F32)
    nc.vector.scalar_tensor_tensor(
        wz[:], w[:], 1.0, Tz[:], ALU.mult, ALU.mult, accum_out=Zb[:]
    )
    wn = pers.tile([BL, P], F32)
    Nb = pers.tile([BL, 1], F32)
    nc.vector.scalar_tensor_tensor(
        wn[:], w[:], 1.0, Te[:], ALU.mult, ALU.mult, accum_out=Nb[:]
    )

    At16 = Ta[:, 0:1]
    Bt16 = Tb[:, 0:1]
    # whole-window log marginal per b
    t1 = pers.tile([BL, 1], F32)
    nc.scalar.activation(t1[:], At16, AF.Square, scale=1.0 / 32.0)
    v1 = pers.tile([BL, 1], F32)
    nc.vector.tensor_scalar_mul(v1[:], Bt16, 1.0 / 1024.0)
    v2 = pers.tile([BL, 1], F32)
    nc.vector.tensor_scalar_mul(v2[:], t1[:], 1.0 / 8192.0)
    vW = pers.tile([BL, 1], F32)
    nc.vector.tensor_sub(vW[:], v1[:], v2[:])
    nc.vector.tensor_scalar(vW[:], vW[:], 1.0 / 8191.0, 1.0e-8, ALU.mult, ALU.max)
    term1 = pers.tile([BL, 1], F32)
    nc.vector.tensor_scalar_mul(term1[:], vW[:], sb(17, BL))
    term2 = pers.tile([BL, 1], F32)
    nc.vector.tensor_scalar_mul(term2[:], t1[:], sb(18, BL))
    uu = pers.tile([BL, 1], F32)
    nc.scalar.activation(uu[:], At16, AF.Identity, bias=sb(6, BL), scale=sb(5, BL))
    u2 = pers.tile([BL, 1], F32)
    nc.scalar.activation(u2[:], uu[:], AF.Square)
    term3 = pers.tile([BL, 1], F32)
    nc.vector.tensor_scalar_mul(term3[:], u2[:], sb(13, BL))
    tsum = pers.tile([BL, 1], F32)
    nc.vector.tensor_add(tsum[:], term1[:], term2[:])
    nc.vector.tensor_sub(tsum[:], tsum[:], term3[:])
    bfW = pers.tile([BL, 1], F32)
    nc.vector.tensor_scalar(bfW[:], tsum[:], -0.5, sb(19, BL), ALU.mult, ALU.add)

    sigin = pers.tile([BL, 1], F32)
    nc.vector.tensor_sub(sigin[:], M16[:], bfW[:])
    sig = pers.tile([BL, 1], F32)
    nc.scalar.activation(sig[:], sigin[:], AF.Sigmoid)
    invZ = pers.tile([BL, 1], F32)
    nc.vector.reciprocal(invZ[:], Zb[:])
    ratio = pers.tile([BL, 1], F32)
    nc.vector.tensor_mul(ratio[:], Nb[:], invZ[:])
    outv = pers.tile([BL, 1], F32)
    nc.vector.tensor_mul(outv[:], sig[:], ratio[:])
    nc.sync.dma_start(out[:], outv[:])


def host_consts():
    gvec = (np.arange(P)[:, None] * U + np.arange(U)[None, :]).astype(np.float32)
    ut = np.triu(np.ones((P, P), np.float32), 1)
    ones = np.ones((P, P), np.float32)
    ident = np.eye(P, dtype=np.float32)
    return gvec, ut, ones, ident


def split_multi_waits(nc):
    """Walrus in this container allows one sync wait per instruction; move
    extra waits onto same-engine NOPs inserted immediately before."""
    import bass_rust

    nid = [0]
    for f in nc.m.functions:
        for b in f.blocks:
            insts = b.instructions
            i = 0
            while i < len(insts):
                ins = insts[i]
                si = ins.sync_info
                if si is not None and si.on_wait is not None and len(si.on_wait) > 1:
                    waits = list(si.on_wait)
                    for w in waits[:-1]:
                        nop = mybir.InstNoOp(
                            name=f"I-waitsplit-{nid[0]}", ins=[], outs=[]
                        )
                        nid[0] += 1
                        nop.engine = ins.engine
                        nop.sync_info = bass_rust.SyncInfo(
                            on_wait=[w], on_update=[]
                        )
                        insts.insert(i, nop)
                        i += 1
                    si.on_wait = waits[-1:]
                i += 1


_NC_CACHE = {}


def build_nc(split=True):
    global _NC_CACHE
    if split in _NC_CACHE:
        return _NC_CACHE[split]
    nc = bass.Bass()
    x = nc.declare_dram_parameter("x", [BL, T, N], F32, isOutput=False)
    params = nc.declare_dram_parameter("params", [P, 3], F32, isOutput=False)
    gvec = nc.declare_dram_parameter("gvec", [P, U], F32, isOutput=False)
    utc = nc.declare_dram_parameter("utc", [P, P], F32, isOutput=False)
    onesc = nc.declare_dram_parameter("onesc", [P, P], F32, isOutput=False)
    idc = nc.declare_dram_parameter("idc", [P, P], F32, isOutput=False)
    out = nc.declare_dram_parameter("out", [BL, 1], F32, isOutput=True)
    with tile.TileContext(nc) as tc:
        with ExitStack() as ctx:
            build_body(ctx, tc, x[:], params[:], gvec[:], utc[:], onesc[:], idc[:], out[:])
    if split:
        split_multi_waits(nc)
    _NC_CACHE[split] = nc
    return nc


def make_in_maps(x, prior_mean, prior_var, noise_var):
    x = np.ascontiguousarray(np.asarray(x, dtype=np.float32))
    params = np.tile(
        np.array(
            [[float(prior_mean[0]), float(prior_var[0]), float(noise_var[0])]],
            dtype=np.float32,
        ),
        (P, 1),
    )
    gvec, ut, ones, ident = host_consts()
    in_maps = []
    for c in range(NCORES):
        in_maps.append(
            {
                "x": x[c * BL : (c + 1) * BL],
                "params": params,
                "gvec": gvec,
                "utc": ut,
                "onesc": ones,
                "idc": ident,
            }
        )
    return in_maps


def kernel(x, prior_mean, prior_var, noise_var):
    from concourse.bass_utils import run_bass_kernel_spmd

    in_maps = make_in_maps(x, prior_mean, prior_var, noise_var)
    nc = build_nc()
    res = run_bass_kernel_spmd(nc, in_maps, list(range(NCORES)))
    outs = [np.asarray(res.results[c]["out"]).reshape(BL) for c in range(NCORES)]
    return np.concatenate(outs).astype(np.float32)


# revision 25
# speedup vs baseline: 1091.8434x; 1091.8434x over previous
"""Trainium2 Bass kernel for BayesianChangePointDetector (segment_reduce).

Contract: kernel(**inputs) takes FULL inputs (x:[128,8192,32] f32, plus 3
scalar prior params) and returns the FULL [128] f32 output. Internally the
batch dim is sharded across 8 NeuronCores (16 rows each, pure data parallel,
no collectives), each core runs the same Bass/Tile program, and the host
concatenates the 8 per-core [16] outputs.

Per-core layout: partition p in [0,128) owns t in [64p, 64p+64); the free dim
is (b, u) with b in [0,16) batch rows and u in [0,64). The heavy pass is a
single DVE reduce over N=32; prefix sums use the native tensor_tensor_scan
plus a cross-partition carry fixed up with a triangular-ones matmul on PE.
"""

import sys

if "/opt/trn_rl_repo" not in sys.path:
    sys.path.insert(0, "/opt/trn_rl_repo")

import math
from contextlib import ExitStack

import numpy as np

import concourse.bass as bass
import concourse.tile as tile
from concourse import mybir

F32 = mybir.dt.float32
AF = mybir.ActivationFunctionType
ALU = mybir.AluOpType
AX = mybir.AxisListType

B, T, N = 128, 8192, 32
NCORES = 8
BL = B // NCORES  # 16 batch rows per core
P = 128           # partitions = t-blocks
U = T // P        # 64 t's per partition
BC = 4            # batch rows per processing chunk
NCHUNK = BL // BC
NS = 32           # scalar-slot count
NEG = -1.0e30

# near-end threshold: P_split > 6553  <=>  g >= 6553 (g = P_split-1 = 64p+u)
NE_P0 = 6553 // U          # 102
NE_U0 = 6553 - NE_P0 * U   # 25
# valid candidates: P_split in [16, 8176) <=> g in [15, 8175)
LO_INV_U = 15              # g<15 -> p==0, u<15 invalid
HI_INV_U = 8174 - 127 * U + 1  # g>8174 -> p==127, u>=47 invalid


def build_body(ctx, tc, x, params, gvec, utc, onesc, idc, out):
    nc = tc.nc
    pers = ctx.enter_context(tc.tile_pool(name="pers", bufs=1))
    xp = ctx.enter_context(tc.tile_pool(name="xp", bufs=2))
    wk = ctx.enter_context(tc.tile_pool(name="wk", bufs=2))
    psp = ctx.enter_context(tc.tile_pool(name="psp", bufs=2, space="PSUM"))
    ps1 = ctx.enter_context(tc.tile_pool(name="ps1", bufs=1, space="PSUM"))

    # ---------- persistent tiles ----------
    ut_t = pers.tile([P, P], F32)     # strictly-upper triangular ones (q<m)
    ones_t = pers.tile([P, P], F32)   # all-ones
    id_t = pers.tile([P, P], F32)     # identity (PE transpose)
    gt = pers.tile([P, U], F32)       # g = 64p+u
    nc.sync.dma_start(ut_t[:], utc[:])
    nc.sync.dma_start(ones_t[:], onesc[:])
    nc.sync.dma_start(id_t[:], idc[:])
    nc.sync.dma_start(gt[:], gvec[:])

    ptile = pers.tile([P, 3], F32)
    nc.sync.dma_start(ptile[:], params[:])

    # scalar slots, computed redundantly on all 128 partitions
    sv = pers.tile([P, NS], F32)
    tmp = pers.tile([P, 8], F32)

    def s(i):
        return sv[:, i : i + 1]

    def tm(i):
        return tmp[:, i : i + 1]

    # ---------- scalar prep on partition 0 ----------
    # slots: 0 pm, 1 inv_nv, 2 inv_pv, 3 neg_inv_nv, 4 zRb, 5 k, 6 c,
    # 7 -kq/2, 8 k^2/2, 9 kq/2, 10 c*k, 11 c^2/2, 12 sc, 13 pvW,
    # 14 L2pinv, 15 Lpv, 16 LpvW, 17 8192*inv_nv, 18 inv_nv/8192,
    # 19 bfWc, 20 pv, 21 nv, 22 pm^2*inv_pv, 23 -4096*L2pinv
    # softplus(x) = ln(1 + exp(x)); Exp+Ln share one ACT table set
    nc.scalar.activation(tm(0), ptile[:, 1:2], AF.Exp)
    nc.vector.tensor_scalar_add(tm(0), tm(0), 1.0)
    nc.scalar.activation(s(20), tm(0), AF.Ln)
    nc.scalar.activation(tm(1), ptile[:, 2:3], AF.Exp)
    nc.vector.tensor_scalar_add(tm(1), tm(1), 1.0)
    nc.scalar.activation(s(21), tm(1), AF.Ln)
    nc.vector.tensor_copy(s(0), ptile[:, 0:1])
    nc.vector.reciprocal(s(1), s(21))
    nc.vector.reciprocal(s(2), s(20))
    nc.vector.tensor_scalar_mul(s(3), s(1), -1.0)
    nc.vector.tensor_scalar(s(4), s(1), 8191.0, s(2), ALU.mult, ALU.add)
    nc.vector.tensor_scalar_mul(s(5), s(1), 1.0 / 32.0)
    nc.vector.tensor_mul(s(6), s(0), s(2))
    nc.vector.tensor_scalar_mul(s(7), s(1), -0.5 / 1024.0)
    nc.vector.tensor_scalar_mul(s(9), s(1), 0.5 / 1024.0)
    nc.vector.tensor_mul(tm(0), s(5), s(5))
    nc.vector.tensor_scalar_mul(s(8), tm(0), 0.5)
    nc.vector.tensor_mul(s(10), s(6), s(5))
    nc.vector.tensor_mul(tm(1), s(6), s(6))
    nc.vector.tensor_scalar_mul(s(11), tm(1), 0.5)
    nc.scalar.activation(s(14), s(21), AF.Ln, scale=2.0 * math.pi)
    nc.scalar.activation(s(15), s(20), AF.Ln)
    nc.vector.tensor_scalar_mul(s(17), s(1), 8192.0)
    nc.vector.tensor_scalar(tm(2), s(1), 8192.0, s(2), ALU.mult, ALU.add)
    nc.vector.reciprocal(s(13), tm(2))
    nc.scalar.activation(s(16), s(13), AF.Ln)
    nc.vector.tensor_scalar_mul(s(18), s(1), 1.0 / 8192.0)
    nc.vector.tensor_mul(tm(3), s(0), s(0))
    nc.vector.tensor_mul(s(22), tm(3), s(2))
    nc.vector.tensor_scalar_mul(s(23), s(14), -4096.0)
    nc.vector.tensor_sub(tm(4), s(23), s(15))
    nc.vector.tensor_sub(s(12), tm(4), s(22))
    nc.vector.tensor_sub(tm(5), s(16), s(15))
    nc.vector.tensor_scalar_mul(tm(5), tm(5), 0.5)
    nc.vector.tensor_add(tm(6), s(23), tm(5))
    nc.vector.tensor_scalar_mul(tm(7), s(22), -0.5)
    nc.vector.tensor_add(s(19), tm(6), tm(7))

    def sb(i, np_=P, p0=0):
        return sv[p0 : p0 + np_, i : i + 1]

    # ---------- per-candidate coefficient vectors [P, U] ----------
    nf = pers.tile([P, U], F32)
    nc.vector.tensor_scalar_add(nf[:], gt[:], 1.0)
    zL = pers.tile([P, U], F32)
    nc.vector.tensor_scalar(zL[:], nf[:], sb(1), sb(2), ALU.mult, ALU.add)
    pvnL = pers.tile([P, U], F32)
    nc.vector.reciprocal(pvnL[:], zL[:])
    zR = pers.tile([P, U], F32)
    nc.vector.tensor_scalar(zR[:], gt[:], sb(3), sb(4), ALU.mult, ALU.add)
    pvnR = pers.tile([P, U], F32)
    nc.vector.reciprocal(pvnR[:], zR[:])
    lpvnL = pers.tile([P, U], F32)
    nc.scalar.activation(lpvnL[:], pvnL[:], AF.Ln)
    lpvnR = pers.tile([P, U], F32)
    nc.scalar.activation(lpvnR[:], pvnR[:], AF.Ln)
    kc2 = pers.tile([P, U], F32)
    nc.vector.tensor_add(kc2[:], lpvnL[:], lpvnR[:])

    nRf = pers.tile([P, U], F32)
    nc.vector.tensor_scalar(nRf[:], gt[:], -1.0, 8191.0, ALU.mult, ALU.add)
    gc = pers.tile([P, U], F32)
    nc.vector.tensor_scalar_max(gc[:], gt[:], 1.0)
    inv_n1 = pers.tile([P, U], F32)
    nc.vector.reciprocal(inv_n1[:], gc[:])
    nR1c = pers.tile([P, U], F32)
    nc.vector.tensor_scalar(nR1c[:], gt[:], -1.0, 8190.0, ALU.mult, ALU.add)
    nc.vector.tensor_scalar_max(nR1c[:], nR1c[:], 1.0)
    inv_nR1 = pers.tile([P, U], F32)
    nc.vector.reciprocal(inv_nR1[:], nR1c[:])
    inv_n = pers.tile([P, U], F32)
    nc.vector.reciprocal(inv_n[:], nf[:])
    inv_nR = pers.tile([P, U], F32)
    nRc = pers.tile([P, U], F32)
    nc.vector.tensor_scalar_max(nRc[:], nRf[:], 1.0)
    nc.vector.reciprocal(inv_nR[:], nRc[:])

    n_n1 = pers.tile([P, U], F32)
    nc.vector.tensor_mul(n_n1[:], nf[:], inv_n1[:])
    nR_nR1 = pers.tile([P, U], F32)
    nc.vector.tensor_mul(nR_nR1[:], nRf[:], inv_nR1[:])
    i_nn1 = pers.tile([P, U], F32)
    nc.vector.tensor_mul(i_nn1[:], inv_n[:], inv_n1[:])
    i_nRnR1 = pers.tile([P, U], F32)
    nc.vector.tensor_mul(i_nRnR1[:], inv_nR[:], inv_nR1[:])

    CBL = pers.tile([P, U], F32)
    nc.scalar.activation(CBL[:], n_n1[:], AF.Copy, scale=sb(7))
    CBR = pers.tile([P, U], F32)
    nc.scalar.activation(CBR[:], nR_nR1[:], AF.Copy, scale=sb(7))
    # CA2L = 0.5*kq*i_nn1 + 0.5*k^2*pvnL
    CA2L = pers.tile([P, U], F32)
    q1 = pers.tile([P, U], F32)
    nc.scalar.activation(q1[:], pvnL[:], AF.Copy, scale=sb(8))
    q2 = pers.tile([P, U], F32)
    nc.scalar.activation(q2[:], i_nn1[:], AF.Copy, scale=sb(9))
    nc.vector.tensor_add(CA2L[:], q1[:], q2[:])
    CA2R = pers.tile([P, U], F32)
    q1b = pers.tile([P, U], F32)
    nc.scalar.activation(q1b[:], pvnR[:], AF.Copy, scale=sb(8))
    q2b = pers.tile([P, U], F32)
    nc.scalar.activation(q2b[:], i_nRnR1[:], AF.Copy, scale=sb(9))
    nc.vector.tensor_add(CA2R[:], q1b[:], q2b[:])
    CAL = pers.tile([P, U], F32)
    nc.scalar.activation(CAL[:], pvnL[:], AF.Copy, scale=sb(10))
    CAR = pers.tile([P, U], F32)
    nc.scalar.activation(CAR[:], pvnR[:], AF.Copy, scale=sb(10))
    Cc = pers.tile([P, U], F32)
    p12 = pers.tile([P, U], F32)
    nc.vector.tensor_add(p12[:], pvnL[:], pvnR[:])
    cc1 = pers.tile([P, U], F32)
    nc.scalar.activation(cc1[:], p12[:], AF.Copy, scale=sb(11))
    cct = pers.tile([P, U], F32)
    nc.vector.tensor_scalar(cct[:], kc2[:], 0.5, sb(12), ALU.mult, ALU.add)
    nc.vector.tensor_add(Cc[:], cc1[:], cct[:])
    # bake the invalid-candidate mask into Cc: bf = ... + Cc ~ -1e30 there.
    # valid g in [15, 8175); compute via two is_ge comparisons on gt.
    mlo = pers.tile([P, U], F32)
    nc.vector.tensor_scalar(mlo[:], gt[:], 14.5, NEG, ALU.is_lt, ALU.mult)
    mhi = pers.tile([P, U], F32)
    nc.vector.tensor_scalar(mhi[:], gt[:], 8174.5, NEG, ALU.is_ge, ALU.mult)
    nc.vector.tensor_add(Cc[:], Cc[:], mlo[:])
    nc.vector.tensor_add(Cc[:], Cc[:], mhi[:])
    # near-end 0/1 mask (g >= 6553)
    nemask = pers.tile([P, U], F32)
    nc.vector.tensor_scalar(nemask[:], gt[:], 6552.5, None, ALU.is_ge)

    # ---------- persistent accumulators ----------
    bund = pers.tile([P, 80], F32)  # [0:16) rmax | [16:32) Zp | [32:48) En | [48:64) At | [64:80) Bt
    zeros = pers.tile([P, BC * U], F32)
    nc.gpsimd.memset(zeros[:], 0.0)

    # ---------- per-chunk pipeline ----------
    for ci in range(NCHUNK):
        bs = ci * BC
        xt = xp.tile([P, BC, U, N], F32)
        src = x[bs : bs + BC].rearrange("b (p u) n -> p b u n", p=P)
        nc.sync.dma_start(xt[:], src)

        sr = wk.tile([P, BC, U], F32)
        nc.vector.tensor_reduce(sr[:], xt[:].rearrange("p b u n -> p (b u) n"), AX.X, ALU.add)
        sq = wk.tile([P, BC, U], F32)
        nc.scalar.activation(sq[:], sr[:], AF.Square)

        A = wk.tile([P, BC, U], F32)
        nc.vector.tensor_tensor_scan(
            A[:].rearrange("p b u -> p (b u)"),
            sr[:].rearrange("p b u -> p (b u)"),
            zeros[:],
            0.0,
            ALU.add,
            ALU.add,
        )
        Bt_ = wk.tile([P, BC, U], F32)
        nc.vector.tensor_tensor_scan(
            Bt_[:].rearrange("p b u -> p (b u)"),
            sq[:].rearrange("p b u -> p (b u)"),
            zeros[:],
            0.0,
            ALU.add,
            ALU.add,
        )

        # carry fix: rowprev, chunk totals, triangular matmul
        rv = wk.tile([P, 2 * BC], F32)  # [0:BC) rvA | [BC:2BC) rvB
        nc.gpsimd.memset(rv[:, 0:1], 0.0)
        nc.gpsimd.memset(rv[:, BC : BC + 1], 0.0)
        nc.vector.tensor_copy(rv[:, 1:BC], A[:, 0 : BC - 1, U - 1])
        nc.vector.tensor_copy(rv[:, BC + 1 : 2 * BC], Bt_[:, 0 : BC - 1, U - 1])
        ct = wk.tile([P, 2 * BC], F32)
        nc.vector.tensor_sub(ct[:, 0:BC], A[:, :, U - 1], rv[:, 0:BC])
        nc.vector.tensor_sub(ct[:, BC : 2 * BC], Bt_[:, :, U - 1], rv[:, BC : 2 * BC])
        g_ps = psp.tile([P, 2 * BC], F32)
        nc.tensor.matmul(g_ps[:], ut_t[:], ct[:])
        tot_ps = psp.tile([P, 2 * BC], F32)
        nc.tensor.matmul(tot_ps[:], ones_t[:], ct[:])
        off = wk.tile([P, 2 * BC], F32)
        nc.vector.tensor_sub(off[:], g_ps[:], rv[:])

        offA_b = off[:, 0:BC].unsqueeze(2).broadcast_to([P, BC, U])
        offB_b = off[:, BC : 2 * BC].unsqueeze(2).broadcast_to([P, BC, U])
        nc.vector.tensor_add(A[:], A[:], offA_b)
        nc.vector.tensor_add(Bt_[:], Bt_[:], offB_b)

        At_b = tot_ps[:, 0:BC].unsqueeze(2).broadcast_to([P, BC, U])
        Btot_b = tot_ps[:, BC : 2 * BC].unsqueeze(2).broadcast_to([P, BC, U])
        AR = wk.tile([P, BC, U], F32)
        nc.vector.scalar_tensor_tensor(AR[:], A[:], -1.0, At_b, ALU.mult, ALU.add)
        BR = wk.tile([P, BC, U], F32)
        nc.vector.scalar_tensor_tensor(BR[:], Bt_[:], -1.0, Btot_b, ALU.mult, ALU.add)

        A2 = wk.tile([P, BC, U], F32)
        nc.scalar.activation(A2[:], A[:], AF.Square)
        AR2 = wk.tile([P, BC, U], F32)
        nc.scalar.activation(AR2[:], AR[:], AF.Square)

        def cb(t):
            return t[:].unsqueeze(1).broadcast_to([P, BC, U])

        bf = wk.tile([P, BC, U], F32)
        p2 = wk.tile([P, BC, U], F32)
        p3 = wk.tile([P, BC, U], F32)
        p4 = wk.tile([P, BC, U], F32)
        p5 = wk.tile([P, BC, U], F32)
        p6 = wk.tile([P, BC, U], F32)
        nc.vector.tensor_mul(bf[:], A[:], cb(CAL))
        nc.vector.tensor_mul(p2[:], A2[:], cb(CA2L))
        nc.vector.tensor_mul(p3[:], Bt_[:], cb(CBL))
        nc.vector.tensor_mul(p4[:], AR[:], cb(CAR))
        nc.vector.tensor_mul(p5[:], AR2[:], cb(CA2R))
        nc.vector.tensor_mul(p6[:], BR[:], cb(CBR))
        nc.gpsimd.tensor_add(bf[:], bf[:], p2[:])
        nc.gpsimd.tensor_add(bf[:], bf[:], p3[:])
        nc.gpsimd.tensor_add(bf[:], bf[:], p4[:])
        nc.gpsimd.tensor_add(bf[:], bf[:], p5[:])
        nc.gpsimd.tensor_add(bf[:], bf[:], p6[:])
        nc.gpsimd.tensor_add(bf[:], bf[:], cb(Cc))

        # per-(p,b) max, exp with shift, partial sums
        nc.vector.tensor_reduce(bund[:, bs : bs + BC], bf[:], AX.X, ALU.max)
        negr = wk.tile([P, BC], F32)
        nc.vector.tensor_scalar_mul(negr[:], bund[:, bs : bs + BC], -1.0)
        e = wk.tile([P, BC, U], F32)
        escr = wk.tile([P, U], F32)
        for b in range(BC):
            nc.scalar.activation(
                e[:, b, :],
                bf[:, b, :],
                AF.Exp,
                bias=negr[:, b : b + 1],
                accum_out=bund[:, 16 + bs + b : 17 + bs + b],
            )
            # near-end partial sum: sum_u e * nemask
            nc.vector.scalar_tensor_tensor(
                escr[:],
                e[:, b, :],
                1.0,
                nemask[:],
                ALU.mult,
                ALU.mult,
                accum_out=bund[:, 32 + bs + b : 33 + bs + b],
            )
        nc.scalar.copy(bund[:, 48 + bs : 48 + bs + BC], tot_ps[:, 0:BC])
        nc.scalar.copy(bund[:, 64 + bs : 64 + bs + BC], tot_ps[:, BC : 2 * BC])

    # ---------- finale: per-group PE transposes (keep partition base 0) ----------
    Tall = ps1.tile([BL, 5, P], F32)
    for k in range(5):
        nc.tensor.transpose(Tall[:, k, :], bund[:, 16 * k : 16 * k + BL], id_t[:])
    Tm = Tall[:, 0, :]
    Tz = Tall[:, 1, :]
    Te = Tall[:, 2, :]
    Ta = Tall[:, 3, :]
    Tb = Tall[:, 4, :]

    M16 = pers.tile([BL, 1], F32)
    nc.vector.tensor_reduce(M16[:], Tm[:], AX.X, ALU.max)
    d = pers.tile([BL, P], F32)
    nc.vector.tensor_scalar_sub(d[:], Tm[:], M16[:])
    w = pers.tile([BL, P], F32)
    nc.scalar.activation(w[:], d[:], AF.Exp)
    wz = pers.tile([BL, P], F32)
    Zb = pers.tile([BL, 1], F32)
    nc.vector.scalar_tensor_tensor(
        wz[:], w[:], 1.0, Tz[:], ALU.mult, ALU.mult, accum_out=Zb[:]
    )
    wn = pers.tile([BL, P], F32)
    Nb = pers.tile([BL, 1], F32)
    nc.vector.scalar_tensor_tensor(
        wn[:], w[:], 1.0, Te[:], ALU.mult, ALU.mult, accum_out=Nb[:]
    )

    At16 = Ta[:, 0:1]
    Bt16 = Tb[:, 0:1]
    # whole-window log marginal per b
    t1 = pers.tile([BL, 1], F32)
    nc.scalar.activation(t1[:], At16, AF.Square, scale=1.0 / 32.0)
    v1 = pers.tile([BL, 1], F32)
    nc.vector.tensor_scalar_mul(v1[:], Bt16, 1.0 / 1024.0)
    v2 = pers.tile([BL, 1], F32)
    nc.vector.tensor_scalar_mul(v2[:], t1[:], 1.0 / 8192.0)
    vW = pers.tile([BL, 1], F32)
    nc.vector.tensor_sub(vW[:], v1[:], v2[:])
    nc.vector.tensor_scalar(vW[:], vW[:], 1.0 / 8191.0, 1.0e-8, ALU.mult, ALU.max)
    term1 = pers.tile([BL, 1], F32)
    nc.vector.tensor_scalar_mul(term1[:], vW[:], sb(17, BL))
    term2 = pers.tile([BL, 1], F32)
    nc.vector.tensor_scalar_mul(term2[:], t1[:], sb(18, BL))
    uu = pers.tile([BL, 1], F32)
    nc.scalar.activation(uu[:], At16, AF.Identity, bias=sb(6, BL), scale=sb(5, BL))
    u2 = pers.tile([BL, 1], F32)
    nc.scalar.activation(u2[:], uu[:], AF.Square)
    term3 = pers.tile([BL, 1], F32)
    nc.vector.tensor_scalar_mul(term3[:], u2[:], sb(13, BL))
    tsum = pers.tile([BL, 1], F32)
    nc.vector.tensor_add(tsum[:], term1[:], term2[:])
    nc.vector.tensor_sub(tsum[:], tsum[:], term3[:])
    bfW = pers.tile([BL, 1], F32)
    nc.vector.tensor_scalar(bfW[:], tsum[:], -0.5, sb(19, BL), ALU.mult, ALU.add)

    sigin = pers.tile([BL, 1], F32)
    nc.vector.tensor_sub(sigin[:], M16[:], bfW[:])
    sig = pers.tile([BL, 1], F32)
    nc.scalar.activation(sig[:], sigin[:], AF.Sigmoid)
    invZ = pers.tile([BL, 1], F32)
    nc.vector.reciprocal(invZ[:], Zb[:])
    ratio = pers.tile([BL, 1], F32)
    nc.vector.tensor_mul(ratio[:], Nb[:], invZ[:])
    outv = pers.tile([BL, 1], F32)
    nc.vector.tensor_mul(outv[:], sig[:], ratio[:])
    nc.sync.dma_start(out[:], outv[:])


def host_consts():
    gvec = (np.arange(P)[:, None] * U + np.arange(U)[None, :]).astype(np.float32)
    ut = np.triu(np.ones((P, P), np.float32), 1)
    ones = np.ones((P, P), np.float32)
    ident = np.eye(P, dtype=np.float32)
    return gvec, ut, ones, ident


def split_multi_waits(nc):
    """Walrus in this container allows one sync wait per instruction; move
    extra waits onto same-engine NOPs inserted immediately before."""
    import bass_rust

    nid = [0]
    for f in nc.m.functions:
        for b in f.blocks:
            insts = b.instructions
            i = 0
            while i < len(insts):
                ins = insts[i]
                si = ins.sync_info
                if si is not None and si.on_wait is not None and len(si.on_wait) > 1:
                    waits = list(si.on_wait)
                    for w in waits[:-1]:
                        nop = mybir.InstNoOp(
                            name=f"I-waitsplit-{nid[0]}", ins=[], outs=[]
                        )
                        nid[0] += 1
                        nop.engine = ins.engine
                        nop.sync_info = bass_rust.SyncInfo(
                            on_wait=[w], on_update=[]
                        )
                        insts.insert(i, nop)
                        i += 1
                    si.on_wait = waits[-1:]
                i += 1


_NC_CACHE = {}


def build_nc(split=True, reps=1):
    global _NC_CACHE
    key = (split, reps)
    if key in _NC_CACHE:
        return _NC_CACHE[key]
    nc = bass.Bass()
    x = nc.declare_dram_parameter("x", [BL, T, N], F32, isOutput=False)
    params = nc.declare_dram_parameter("params", [P, 3], F32, isOutput=False)
    gvec = nc.declare_dram_parameter("gvec", [P, U], F32, isOutput=False)
    utc = nc.declare_dram_parameter("utc", [P, P], F32, isOutput=False)
    onesc = nc.declare_dram_parameter("onesc", [P, P], F32, isOutput=False)
    idc = nc.declare_dram_parameter("idc", [P, P], F32, isOutput=False)
    out = nc.declare_dram_parameter("out", [BL, 1], F32, isOutput=True)
    with tile.TileContext(nc) as tc:
        for _ in range(reps):
            with ExitStack() as ctx:
                build_body(
                    ctx, tc, x[:], params[:], gvec[:], utc[:], onesc[:], idc[:], out[:]
                )
    if split:
        split_multi_waits(nc)
    _NC_CACHE[key] = nc
    return nc


def make_in_maps(x, prior_mean, prior_var, noise_var):
    x = np.ascontiguousarray(np.asarray(x, dtype=np.float32))
    params = np.tile(
        np.array(
            [[float(prior_mean[0]), float(prior_var[0]), float(noise_var[0])]],
            dtype=np.float32,
        ),
        (P, 1),
    )
    gvec, ut, ones, ident = host_consts()
    in_maps = []
    for c in range(NCORES):
        in_maps.append(
            {
                "x": x[c * BL : (c + 1) * BL],
                "params": params,
                "gvec": gvec,
                "utc": ut,
                "onesc": ones,
                "idc": ident,
            }
        )
    return in_maps


def kernel(x, prior_mean, prior_var, noise_var):
    from concourse.bass_utils import run_bass_kernel_spmd

    in_maps = make_in_maps(x, prior_mean, prior_var, noise_var)
    nc = build_nc()
    res = run_bass_kernel_spmd(nc, in_maps, list(range(NCORES)))
    outs = [np.asarray(res.results[c]["out"]).reshape(BL) for c in range(NCORES)]
    return np.concatenate(outs).astype(np.float32)
